# revision 2
# baseline (speedup 1.0000x reference)
"""Multi-head causal self-attention (B=4, T=2048, C=1024, H=16, D=64) on 8
Trainium2 NeuronCores.

Sharding: tensor-parallel over heads — 2 heads per core. Each core computes
q/k/v projections for its 2 heads, causal attention, and its row-slice of the
output projection (partial sums over its heads' 128 rows of Wp). The host
sums the 8 partial projections (the "all-reduce") and adds the bias.

Matmuls run as float32r (fp32 data, reduced-precision PE fast path: 1
cycle/row at moving-dim >= 256 vs 4 cycles/row for plain fp32).

A post-pass splits excess semaphore waits: this walrus build accepts only ONE
sync wait per instruction for several instruction structs (fused fp32-family
Matmult/LDW, Drain, ACT). Excess waits move onto injected NoOps on the same
engine queue, which preserves semantics (queue executes waits in order).
"""
import functools

import numpy as np

import concourse.bass as bass
import concourse.mybir as mybir
import concourse.tile as tile
from concourse.masks import make_identity

F32 = mybir.dt.float32
F32R = mybir.dt.float32r

C, H, D = 1024, 16, 64
NCORES = 8
HPC = H // NCORES          # heads per core = 2
CK = C // 128              # c-chunks = 8
ActF = mybir.ActivationFunctionType


def split_excess_waits(nc):
    """Move all-but-one sync wait of every instruction onto injected NoOps."""
    n_split = 0
    for f in nc.m.functions:
        for blk in f.blocks:
            out, changed = [], False
            for inst in blk.instructions:
                si = inst.sync_info
                if si is not None and len(si.on_wait) > 1:
                    for w_ in si.on_wait[:-1]:
                        nop = mybir.InstNoOp(name=f"I-wsplit-{n_split}")
                        n_split += 1
                        nop.engine = inst.engine
                        nop.sync_info = mybir.SyncInfo(on_wait=[w_], on_update=[])
                        out.append(nop)
                    inst.sync_info = mybir.SyncInfo(
                        on_wait=si.on_wait[-1:], on_update=si.on_update)
                    changed = True
                out.append(inst)
            if changed:
                blk.instructions = out
    return n_split


def build_nc(B, T):
    """One SPMD program; all 8 cores run it on different weight slices."""
    NIB = T // 512           # i-blocks per sequence
    NJT = T // 128           # j-tiles per sequence
    nc = bass.Bass()

    x_d = nc.dram_tensor("x", [B, T, C], F32R, kind="ExternalInput")
    wqkv_d = nc.dram_tensor("wqkv", [CK, 128, 3, 128], F32R, kind="ExternalInput")
    wp_d = nc.dram_tensor("wp", [128, C], F32R, kind="ExternalInput")
    out_d = nc.dram_tensor("out", [B, T, C], F32, kind="ExternalOutput")
    scratch = [nc.dram_tensor(f"scratch{b}", [HPC, T], F32) for b in range(B)]

    with tile.TileContext(nc) as tc:
        with (
            tc.tile_pool(name="consts", bufs=1) as consts,
            tc.tile_pool(name="xp", bufs=3) as xp,
            tc.tile_pool(name="xtp", bufs=2) as xtp,
            tc.tile_pool(name="qkv", bufs=1) as qkv,
            tc.tile_pool(name="vnp", bufs=2) as vnp,
            tc.tile_pool(name="pp", bufs=4) as ppool,
            tc.tile_pool(name="norm", bufs=1) as norm,
            tc.tile_pool(name="yp", bufs=3) as ypool,
            tc.tile_pool(name="ps_tr", bufs=2, space="PSUM") as ps_tr,
            tc.tile_pool(name="ps_mm", bufs=3, space="PSUM") as ps_mm,
            tc.tile_pool(name="ps_ot", bufs=2, space="PSUM") as ps_ot,
        ):
            ident_f = consts.tile([128, 128], F32)
            make_identity(nc, ident_f)
            ident = consts.tile([128, 128], F32R)
            nc.vector.tensor_copy(ident, ident_f)
            ones_f = consts.tile([128, NJT], F32)
            nc.vector.memset(ones_f, 1.0)

            w_all = consts.tile([128, CK, 3, 128], F32R)
            nc.sync.dma_start(out=w_all, in_=wqkv_d[:].rearrange("k c t f -> c k t f"))
            wp_t = consts.tile([128, C], F32R)
            nc.sync.dma_start(out=wp_t, in_=wp_d[:])

            for b in range(B):
                qT = qkv.tile([128, T], F32R, tag="qT")
                kT = qkv.tile([128, T], F32R, tag="kT")
                vT = qkv.tile([128, T], F32R, tag="vT")

                # ---- stage A+B: x load, transpose, q/k/v projections ----
                for ib in range(NIB):
                    xT = xtp.tile([128, CK, 512], F32R)
                    for jt in range(4):
                        xt = xp.tile([128, C], F32R)
                        r0 = (ib * 4 + jt) * 128
                        nc.sync.dma_start(out=xt, in_=x_d[b, r0:r0 + 128, :])
                        for g in range(2):          # chunk groups of 4
                            tr = ps_tr.tile([128, 4, 128], F32R, tag="tr")
                            for k in range(4):
                                ck = g * 4 + k
                                nc.tensor.transpose(
                                    tr[:, k, :], xt[:, ck * 128:(ck + 1) * 128], ident)
                            nc.vector.tensor_copy(
                                xT[:, g * 4:(g + 1) * 4, jt * 128:(jt + 1) * 128], tr)
                    for t, dest in enumerate((qT, kT, vT)):
                        acc = ps_mm.tile([128, 512], F32, tag="mm")
                        for ck in range(CK):
                            nc.tensor.matmul(acc, w_all[:, ck, t, :], xT[:, ck, :],
                                             start=(ck == 0), stop=(ck == CK - 1))
                        nc.vector.tensor_copy(dest[:, ib * 512:(ib + 1) * 512], acc)

                # ---- v natural layout [j, 65] with ones column ----
                vn = []
                for h in range(HPC):
                    vh = vnp.tile([128, NJT, 65], F32R, tag="vn")
                    vn.append(vh)
                    for g in range(NJT // 4):
                        tr = ps_tr.tile([128, 4, 128], F32R, tag="tr")
                        for k in range(4):
                            jt = g * 4 + k
                            nc.tensor.transpose(
                                tr[:, k, 0:64],
                                vT[h * 64:(h + 1) * 64, jt * 128:(jt + 1) * 128],
                                ident[h * 64:(h + 1) * 64, h * 64:(h + 1) * 64])
                        nc.vector.tensor_copy(vh[:, g * 4:(g + 1) * 4, 0:64],
                                              tr[:, :, 0:64])
                    nc.scalar.activation(vh[:, :, 64], ones_f, ActF.Copy)

                # ---- attention ----
                otu = norm.tile([128, T], F32, tag="otu")       # unnormalized O^T
                den = [norm.tile([1, T], F32, tag=f"den{h}", name=f"den{h}") for h in range(HPC)]
                for ib in range(NIB):
                    njc = 4 * (ib + 1)
                    ot = [ps_ot.tile([65, 512], F32, tag="ot", name=f"ot{_h}") for _h in range(HPC)]
                    for jc in range(njc):
                        for h in range(HPC):
                            sp = ps_mm.tile([128, 512], F32, tag="mm")
                            nc.tensor.matmul(
                                sp,
                                kT[h * 64:(h + 1) * 64, jc * 128:(jc + 1) * 128],
                                qT[h * 64:(h + 1) * 64, ib * 512:(ib + 1) * 512],
                                start=True, stop=True)
                            pt = ppool.tile([128, 512], F32R, tag="p")
                            nc.scalar.activation(pt, sp, ActF.Exp, scale=D ** -0.5)
                            m = jc - 4 * ib
                            if m >= 0:   # diagonal block: zero where j > i
                                nc.gpsimd.affine_select(
                                    out=pt, in_=pt,
                                    compare_op=mybir.AluOpType.is_ge, fill=0.0,
                                    base=-(128 * m), pattern=[[1, 512]],
                                    channel_multiplier=-1)
                            nc.tensor.matmul(ot[h], vn[h][:, jc, :], pt,
                                             start=(jc == 0), stop=(jc == njc - 1))
                    for h in range(HPC):
                        nc.vector.tensor_copy(
                            otu[h * 64:(h + 1) * 64, ib * 512:(ib + 1) * 512],
                            ot[h][0:64, :])
                        nc.vector.tensor_copy(
                            den[h][0:1, ib * 512:(ib + 1) * 512], ot[h][64:65, :])

                # ---- normalization ----
                rec = [norm.tile([1, T], F32, tag=f"rec{h}", name=f"rec{h}") for h in range(HPC)]
                for h in range(HPC):
                    nc.vector.reciprocal(rec[h], den[h])
                    nc.gpsimd.dma_start(out=scratch[b][h:h + 1, :], in_=rec[h])
                rb = norm.tile([128, T], F32, tag="rb")
                for h in range(HPC):
                    src = bass.AP(tensor=scratch[b][:].tensor, offset=h * T,
                                  ap=[[0, 64], [1, T]])
                    nc.gpsimd.dma_start(out=rb[h * 64:(h + 1) * 64, :], in_=src)
                otn = norm.tile([128, T], F32R, tag="otn")
                for ib in range(NIB):
                    s = slice(ib * 512, (ib + 1) * 512)
                    nc.vector.tensor_mul(otn[:, s], otu[:, s], rb[:, s])

                # ---- output projection (partial: this core's 128 rows of Wp) ----
                for it in range(NJT):
                    for cb in range(C // 512):
                        yp = ps_mm.tile([128, 512], F32, tag="mm")
                        nc.tensor.matmul(yp, otn[:, it * 128:(it + 1) * 128],
                                         wp_t[:, cb * 512:(cb + 1) * 512],
                                         start=True, stop=True)
                        ys = ypool.tile([128, 512], F32, tag="y")
                        nc.vector.tensor_copy(ys, yp)
                        nc.sync.dma_start(
                            out=out_d[b, it * 128:(it + 1) * 128,
                                      cb * 512:(cb + 1) * 512],
                            in_=ys)

    split_excess_waits(nc)
    return nc


# ---------------------------------------------------------------------------
# Host-side: sharding, PJRT runner (compiled once per process), gather.
# ---------------------------------------------------------------------------

class _Runner:
    def __init__(self, B, T):
        import jax
        from jax.experimental.shard_map import shard_map
        from jax.sharding import Mesh, PartitionSpec
        from concourse.bass2jax import (_bass_exec_p, install_neuronx_cc_hook,
                                        partition_id_tensor)

        install_neuronx_cc_hook()
        nc = build_nc(B, T)
        self.nc = nc
        in_names, out_names, out_avals, zero_outs = [], [], [], []
        partition_name = (nc.partition_id_tensor.name
                          if nc.partition_id_tensor else None)
        for alloc in nc.m.functions[0].allocations:
            if not isinstance(alloc, mybir.MemoryLocationSet):
                continue
            name = alloc.memorylocations[0].name
            if alloc.kind == "ExternalInput":
                if name != partition_name:
                    in_names.append(name)
            elif alloc.kind == "ExternalOutput":
                out_names.append(name)
                shape = tuple(alloc.tensor_shape)
                dtype = mybir.dt.np(alloc.dtype)
                out_avals.append(jax.core.ShapedArray(shape, dtype))
                zero_outs.append(np.zeros(shape, dtype))
        self.in_names = list(in_names)
        self.out_names = out_names
        self.out_shapes = [tuple(a.shape) for a in out_avals]
        all_in_names = in_names + out_names
        if partition_name is not None:
            all_in_names.append(partition_name)

        def _body(*args):
            operands = list(args)
            if partition_name is not None:
                operands.append(partition_id_tensor())
            outs = _bass_exec_p.bind(
                *operands,
                out_avals=tuple(out_avals),
                in_names=tuple(all_in_names),
                out_names=tuple(out_names),
                lowering_input_output_aliases=(),
                sim_require_finite=True,
                sim_require_nnan=True,
                nc=nc,
            )
            return tuple(outs)

        devices = jax.devices()[:NCORES]
        self.mesh = Mesh(np.asarray(devices), ("core",))
        n_in = len(in_names) + len(out_names)
        self.fn = jax.jit(shard_map(
            _body, mesh=self.mesh,
            in_specs=(PartitionSpec("core"),) * n_in,
            out_specs=(PartitionSpec("core"),) * len(out_names),
            check_rep=False,
        ), keep_unused=True)
        self.zero_outs = zero_outs
        self._jax = jax

    def prepare(self, in_maps):
        """Concat per-core inputs along axis 0 and device_put."""
        jax = self._jax
        from jax.sharding import NamedSharding, PartitionSpec
        sh = NamedSharding(self.mesh, PartitionSpec("core"))
        args = []
        for i, name in enumerate(self.in_names):
            cat = np.concatenate([np.asarray(m[name]) for m in in_maps], axis=0)
            args.append(jax.device_put(cat, sh))
        for z in self.zero_outs:
            zz = np.zeros((NCORES * z.shape[0], *z.shape[1:]), z.dtype)
            args.append(jax.device_put(zz, sh))
        return args

    def run(self, args):
        outs = self.fn(*args)
        self._jax.block_until_ready(outs)
        return outs

    def split_outs(self, outs):
        res = []
        for c in range(NCORES):
            d = {}
            for i, name in enumerate(self.out_names):
                d[name] = np.asarray(outs[i]).reshape(
                    NCORES, *self.out_shapes[i])[c]
            res.append(d)
        return res


@functools.lru_cache(maxsize=2)
def _get_runner(B, T):
    return _Runner(B, T)


def make_in_maps(x, Wq, Wk, Wv, Wp):
    """Per-core input dicts from full tensors."""
    x = np.asarray(x, np.float32)
    Wq, Wk, Wv = (np.asarray(w, np.float32) for w in (Wq, Wk, Wv))
    Wp = np.asarray(Wp, np.float32)
    in_maps = []
    for c in range(NCORES):
        hs = slice(c * HPC, (c + 1) * HPC)
        wqkv = np.stack([Wq[hs], Wk[hs], Wv[hs]])          # [3, HPC, C, D]
        wqkv = wqkv.reshape(3, HPC, CK, 128, D)
        wqkv = wqkv.transpose(2, 3, 0, 1, 4).reshape(CK, 128, 3, HPC * D)
        wp = Wp[c * HPC * D:(c + 1) * HPC * D]             # [128, C]
        in_maps.append({
            "x": x,
            "wqkv": np.ascontiguousarray(wqkv),
            "wp": np.ascontiguousarray(wp),
        })
    return in_maps


def kernel(x, Wq, Wk, Wv, Wp, bp):
    B, T, _ = x.shape
    runner = _get_runner(B, T)
    args = runner.prepare(make_in_maps(x, Wq, Wk, Wv, Wp))
    outs = runner.run(args)
    per_core = runner.split_outs(outs)
    acc = per_core[0]["out"].astype(np.float32)
    for c in range(1, NCORES):
        acc = acc + per_core[c]["out"]
    return (acc + np.asarray(bp, np.float32)).astype(np.float32)


# revision 12
# speedup vs baseline: 99.6842x; 99.6842x over previous
"""Multi-head causal self-attention (B=4, T=2048, C=1024, H=16, D=64) on 8
Trainium2 NeuronCores.

Sharding: tensor-parallel over heads — 2 heads per core. Each core computes
q/k/v projections for its 2 heads, causal attention, and its row-slice of the
output projection (partial sums over its heads' 128 rows of Wp). The host
sums the 8 partial projections (the "all-reduce") and adds the bias. x is
pre-transposed on the host (part of shard prep) so the contraction dim is on
partitions without on-chip transposes.

Matmuls run as float32r (fp32 data, reduced-precision PE fast path: 1
cycle/row at moving-dim >= 256 vs 4 cycles/row for plain fp32).

A post-pass splits excess semaphore waits: this walrus build accepts only ONE
sync wait per instruction for several instruction structs (fused fp32-family
Matmult/LDW, Drain, ACT). Excess waits move onto injected NoOps on the same
engine queue, which preserves semantics (queue executes waits in order).
"""
import functools

import numpy as np

import concourse.bass as bass
import concourse.mybir as mybir
import concourse.tile as tile
from concourse.masks import make_identity

F32 = mybir.dt.float32
F32R = mybir.dt.float32r

C, H, D = 1024, 16, 64
NCORES = 8
HPC = H // NCORES          # heads per core = 2
CK = C // 128              # c-chunks = 8
ActF = mybir.ActivationFunctionType


def split_excess_waits(nc):
    """Move all-but-one sync wait of every instruction onto injected NoOps."""
    n_split = 0
    for f in nc.m.functions:
        for blk in f.blocks:
            out, changed = [], False
            for inst in blk.instructions:
                si = inst.sync_info
                if si is not None and len(si.on_wait) > 1:
                    for w_ in si.on_wait[:-1]:
                        nop = mybir.InstNoOp(name=f"I-wsplit-{n_split}")
                        n_split += 1
                        nop.engine = inst.engine
                        nop.sync_info = mybir.SyncInfo(on_wait=[w_], on_update=[])
                        out.append(nop)
                    inst.sync_info = mybir.SyncInfo(
                        on_wait=si.on_wait[-1:], on_update=si.on_update)
                    changed = True
                out.append(inst)
            if changed:
                blk.instructions = out
    return n_split


def build_nc(B, T):
    """One SPMD program; all 8 cores run it on different weight slices."""
    NIB = T // 512           # i-blocks per sequence
    NJT = T // 128           # j-tiles per sequence
    nc = bass.Bass()

    xt_d = nc.dram_tensor("xt", [C, B, T], F32R, kind="ExternalInput")
    wqkv_d = nc.dram_tensor("wqkv", [CK, 128, 3, 128], F32R, kind="ExternalInput")
    wp_d = nc.dram_tensor("wp", [128, C], F32R, kind="ExternalInput")
    out_d = nc.dram_tensor("out", [B, T, C], F32, kind="ExternalOutput")
    scr = {(b, h): nc.dram_tensor(f"scr{b}_{h}", [1, T], F32)
           for b in range(B) for h in range(HPC)}

    with tile.TileContext(nc) as tc:
        with (
            tc.tile_pool(name="consts", bufs=1) as consts,
            tc.tile_pool(name="xtp", bufs=2) as xtp,
            tc.tile_pool(name="qkv", bufs=2) as qkv,
            tc.tile_pool(name="vnp", bufs=2) as vnp,
            tc.tile_pool(name="pp", bufs=3) as ppool,
            tc.tile_pool(name="nrm", bufs=4) as nrm,
            tc.tile_pool(name="ot", bufs=2) as otp,
            tc.tile_pool(name="yp", bufs=3) as ypool,
            tc.tile_pool(name="ps_mm", bufs=2, space="PSUM") as ps_mm,
            tc.tile_pool(name="ps_ot", bufs=2, space="PSUM") as ps_ot,
        ):
            ident_f = consts.tile([128, 128], F32)
            make_identity(nc, ident_f)
            ident = consts.tile([128, 128], F32R)
            nc.vector.tensor_copy(ident, ident_f)
            ones_f = consts.tile([128, NJT], F32)
            nc.vector.memset(ones_f, 1.0)
            w_all = consts.tile([128, CK, 3, 128], F32R)
            for ck in range(CK):
                nc.sync.dma_start(out=w_all[:, ck, :, :], in_=wqkv_d[ck])
            wp_t = consts.tile([128, C], F32R)
            nc.sync.dma_start(out=wp_t, in_=wp_d[:])

            def make_proj(b):
                """Tiles + emission units for batch b's q/k/v projections.
                Units are closures emitted interleaved into the previous
                batch's attention so the PE queue stays fed while ACT
                drains the exp backlog."""
                tiles = {
                    "qT": qkv.tile([128, T], F32R, tag="qT", name=f"qT{b}"),
                    "kT": qkv.tile([128, T], F32R, tag="kT", name=f"kT{b}"),
                    "vT": qkv.tile([128, T], F32R, tag="vT", name=f"vT{b}"),
                    "vn": [vnp.tile([128, NJT, 65], F32R, tag="vn",
                                    name=f"vn{b}_{h}") for h in range(HPC)],
                }
                units = []
                for ib in range(NIB):
                    s = slice(ib * 512, (ib + 1) * 512)

                    def u_dma(ib=ib, s=s):
                        xT = xtp.tile([128, CK, 512], F32R, name="xT")
                        for ck in range(CK):
                            nc.sync.dma_start(
                                out=xT[:, ck, :],
                                in_=xt_d[ck * 128:(ck + 1) * 128, b, s])
                        tiles[("xT", ib)] = xT
                    units.append(u_dma)
                    for t, key in enumerate(("qT", "kT", "vT")):
                        def u_mm_a(t=t, key=key, ib=ib):
                            xT = tiles[("xT", ib)]
                            acc = ps_ot.tile([128, 512], F32, tag="aux", bufs=2,
                                             name="acc")
                            for ck in range(CK // 2):
                                nc.tensor.matmul(acc, w_all[:, ck, t, :],
                                                 xT[:, ck, :],
                                                 start=(ck == 0), stop=False)
                            tiles[("acc", ib, t)] = acc
                        def u_mm_b(t=t, key=key, ib=ib, s=s):
                            xT = tiles[("xT", ib)]
                            acc = tiles.pop(("acc", ib, t))
                            for ck in range(CK // 2, CK):
                                nc.tensor.matmul(acc, w_all[:, ck, t, :],
                                                 xT[:, ck, :],
                                                 start=False,
                                                 stop=(ck == CK - 1))
                            nc.vector.tensor_copy(tiles[key][:, s], acc)
                        units.append(u_mm_a)
                        units.append(u_mm_b)
                    for h in range(HPC):
                        def u_tr(h=h, ib=ib):
                            vT = tiles["vT"]
                            tr = ps_ot.tile([128, 4, 64], F32R, tag="aux", bufs=2,
                                            name="tr")
                            for k in range(4):
                                jt = ib * 4 + k
                                nc.tensor.transpose(
                                    tr[:, k, :],
                                    vT[h * 64:(h + 1) * 64,
                                       jt * 128:(jt + 1) * 128],
                                    ident[h * 64:(h + 1) * 64,
                                          h * 64:(h + 1) * 64])
                            nc.vector.tensor_copy(
                                tiles["vn"][h][:, ib * 4:(ib + 1) * 4, 0:64], tr)
                        units.append(u_tr)
                def u_ones():
                    for h in range(HPC):
                        nc.scalar.activation(tiles["vn"][h][:, :, 64], ones_f,
                                             ActF.Copy)
                units.append(u_ones)
                return tiles, units

            def attention(b, tiles, interleave):
                """Attention for batch b; pops `interleave` units between jc
                iterations. Inner loop software-pipelined (MM2 one jc behind
                MM1/exp); yproj delayed one ib behind normalization."""
                qT, kT, vn = tiles["qT"], tiles["kT"], tiles["vn"]
                otn = otp.tile([128, T], F32R, tag="otn", name=f"otn{b}")
                n_jc_total = sum(4 * (ib + 1) for ib in range(NIB))
                jc_done = 0
                emitted = 0
                units = list(interleave)

                def pace():
                    nonlocal emitted
                    want = (jc_done * len(units)) // max(n_jc_total - 2, 1)
                    while emitted < min(want, len(units)):
                        units[emitted]()
                        emitted += 1

                def emit_yproj(ib):
                    for k in range(4):
                        it = ib * 4 + k
                        for cb in range(C // 512):
                            yp = ps_ot.tile([128, 512], F32, tag="aux", bufs=2,
                                            name="yp")
                            nc.tensor.matmul(yp, otn[:, it * 128:(it + 1) * 128],
                                             wp_t[:, cb * 512:(cb + 1) * 512],
                                             start=True, stop=True)
                            ys = ypool.tile([128, 512], F32, tag="y", name="ys")
                            nc.vector.tensor_copy(ys, yp)
                            nc.sync.dma_start(
                                out=out_d[b, it * 128:(it + 1) * 128,
                                          cb * 512:(cb + 1) * 512],
                                in_=ys)

                for ib in range(NIB):
                    s = slice(ib * 512, (ib + 1) * 512)
                    njc = 4 * (ib + 1)
                    ot = [ps_ot.tile([65, 512], F32, tag="ot", name=f"ot{_h}")
                          for _h in range(HPC)]

                    def mm1(jc):
                        s2 = ps_mm.tile([128, 2, 512], F32, tag="s2", name="s2")
                        for h in range(HPC):
                            nc.tensor.matmul(
                                s2[:, h, :],
                                kT[h * 64:(h + 1) * 64, jc * 128:(jc + 1) * 128],
                                qT[h * 64:(h + 1) * 64, s],
                                start=True, stop=True)
                        pt2 = ppool.tile([128, 2, 512], F32R, tag="p", name="pt2")
                        nc.scalar.activation(pt2, s2, ActF.Exp, scale=D ** -0.5)
                        m = jc - 4 * ib
                        if m >= 0:   # diagonal block: zero where j > i
                            for h in range(HPC):
                                nc.gpsimd.affine_select(
                                    out=pt2[:, h, :], in_=pt2[:, h, :],
                                    compare_op=mybir.AluOpType.is_ge, fill=0.0,
                                    base=-(128 * m), pattern=[[1, 512]],
                                    channel_multiplier=-1)
                        return pt2

                    def mm2(jc, pt2):
                        for h in range(HPC):
                            nc.tensor.matmul(ot[h], vn[h][:, jc, :],
                                             pt2[:, h, :],
                                             start=(jc == 0),
                                             stop=(jc == njc - 1))

                    prev = mm1(0)
                    jc_done += 1
                    pace()
                    for jc in range(1, njc):
                        cur = mm1(jc)
                        mm2(jc - 1, prev)
                        prev = cur
                        jc_done += 1
                        pace()
                    mm2(njc - 1, prev)

                    # normalization: reciprocal straight off the psum denom
                    # row; broadcast via DRAM bounce (off the PE queue — the
                    # one-ib yproj delay hides the DMA latency)
                    for h in range(HPC):
                        otu = nrm.tile([64, 512], F32, tag="otu", name="otu")
                        nc.scalar.activation(otu, ot[h][0:64, :], ActF.Copy)
                        rec = nrm.tile([1, 512], F32, tag="rec", name="rec")
                        nc.vector.reciprocal(rec, ot[h][64:65, :])
                        nc.gpsimd.dma_start(out=scr[(b, h)][0:1, s], in_=rec)
                        rb = nrm.tile([64, 512], F32, tag="rb", name="rb")
                        src = bass.AP(tensor=scr[(b, h)][:].tensor,
                                      offset=ib * 512, ap=[[0, 64], [1, 512]])
                        nc.gpsimd.dma_start(out=rb, in_=src)
                        nc.vector.tensor_mul(otn[h * 64:(h + 1) * 64, s],
                                             otu, rb)
                    if ib > 0:
                        emit_yproj(ib - 1)
                # drain any unemitted interleave units, then last yproj
                while emitted < len(units):
                    units[emitted]()
                    emitted += 1
                emit_yproj(NIB - 1)

            cur_tiles, cur_units = make_proj(0)
            for u in cur_units:
                u()
            for b in range(B):
                if b + 1 < B:
                    nxt_tiles, nxt_units = make_proj(b + 1)
                else:
                    nxt_tiles, nxt_units = None, []
                attention(b, cur_tiles, nxt_units)
                cur_tiles = nxt_tiles

    split_excess_waits(nc)
    return nc


# ---------------------------------------------------------------------------
# Host-side: sharding, PJRT runner (compiled once per process), gather.
# ---------------------------------------------------------------------------

class _Runner:
    def __init__(self, B, T):
        import jax
        from jax.experimental.shard_map import shard_map
        from jax.sharding import Mesh, PartitionSpec
        from concourse.bass2jax import (_bass_exec_p, install_neuronx_cc_hook,
                                        partition_id_tensor)

        install_neuronx_cc_hook()
        nc = build_nc(B, T)
        self.nc = nc
        in_names, out_names, out_avals, zero_outs = [], [], [], []
        partition_name = (nc.partition_id_tensor.name
                          if nc.partition_id_tensor else None)
        for alloc in nc.m.functions[0].allocations:
            if not isinstance(alloc, mybir.MemoryLocationSet):
                continue
            name = alloc.memorylocations[0].name
            if alloc.kind == "ExternalInput":
                if name != partition_name:
                    in_names.append(name)
            elif alloc.kind == "ExternalOutput":
                out_names.append(name)
                shape = tuple(alloc.tensor_shape)
                dtype = mybir.dt.np(alloc.dtype)
                out_avals.append(jax.core.ShapedArray(shape, dtype))
                zero_outs.append(np.zeros(shape, dtype))
        self.in_names = list(in_names)
        self.out_names = out_names
        self.out_shapes = [tuple(a.shape) for a in out_avals]
        all_in_names = in_names + out_names
        if partition_name is not None:
            all_in_names.append(partition_name)

        def _body(*args):
            operands = list(args)
            if partition_name is not None:
                operands.append(partition_id_tensor())
            outs = _bass_exec_p.bind(
                *operands,
                out_avals=tuple(out_avals),
                in_names=tuple(all_in_names),
                out_names=tuple(out_names),
                lowering_input_output_aliases=(),
                sim_require_finite=True,
                sim_require_nnan=True,
                nc=nc,
            )
            return tuple(outs)

        devices = jax.devices()[:NCORES]
        self.mesh = Mesh(np.asarray(devices), ("core",))
        n_in = len(in_names) + len(out_names)
        self.fn = jax.jit(shard_map(
            _body, mesh=self.mesh,
            in_specs=(PartitionSpec("core"),) * n_in,
            out_specs=(PartitionSpec("core"),) * len(out_names),
            check_rep=False,
        ), keep_unused=True)
        self.zero_outs = zero_outs
        self._jax = jax

    def prepare(self, in_maps):
        """Concat per-core inputs along axis 0 and device_put."""
        jax = self._jax
        from jax.sharding import NamedSharding, PartitionSpec
        sh = NamedSharding(self.mesh, PartitionSpec("core"))
        args = []
        for i, name in enumerate(self.in_names):
            cat = np.concatenate([np.asarray(m[name]) for m in in_maps], axis=0)
            args.append(jax.device_put(cat, sh))
        for z in self.zero_outs:
            zz = np.zeros((NCORES * z.shape[0], *z.shape[1:]), z.dtype)
            args.append(jax.device_put(zz, sh))
        return args

    def run(self, args):
        outs = self.fn(*args)
        self._jax.block_until_ready(outs)
        return outs

    def split_outs(self, outs):
        res = []
        for c in range(NCORES):
            d = {}
            for i, name in enumerate(self.out_names):
                d[name] = np.asarray(outs[i]).reshape(
                    NCORES, *self.out_shapes[i])[c]
            res.append(d)
        return res


@functools.lru_cache(maxsize=2)
def _get_runner(B, T):
    return _Runner(B, T)


def make_in_maps(x, Wq, Wk, Wv, Wp):
    """Per-core input dicts from full tensors (host-side shard prep)."""
    x = np.asarray(x, np.float32)
    Wq, Wk, Wv = (np.asarray(w, np.float32) for w in (Wq, Wk, Wv))
    Wp = np.asarray(Wp, np.float32)
    xt = np.ascontiguousarray(x.transpose(2, 0, 1))       # [C, B, T]
    in_maps = []
    for c in range(NCORES):
        hs = slice(c * HPC, (c + 1) * HPC)
        wqkv = np.stack([Wq[hs], Wk[hs], Wv[hs]])          # [3, HPC, C, D]
        wqkv = wqkv.reshape(3, HPC, CK, 128, D)
        wqkv = wqkv.transpose(2, 3, 0, 1, 4).reshape(CK, 128, 3, HPC * D)
        wp = Wp[c * HPC * D:(c + 1) * HPC * D]             # [128, C]
        in_maps.append({
            "xt": xt,
            "wqkv": np.ascontiguousarray(wqkv),
            "wp": np.ascontiguousarray(wp),
        })
    return in_maps


def kernel(x, Wq, Wk, Wv, Wp, bp):
    B, T, _ = x.shape
    runner = _get_runner(B, T)
    args = runner.prepare(make_in_maps(x, Wq, Wk, Wv, Wp))
    outs = runner.run(args)
    per_core = runner.split_outs(outs)
    acc = per_core[0]["out"].astype(np.float32)
    for c in range(1, NCORES):
        acc = acc + per_core[c]["out"]
    return (acc + np.asarray(bp, np.float32)).astype(np.float32)
